# revision 3
# baseline (speedup 1.0000x reference)
"""Trainium2 Bass kernel for nn_Attention_27255862460439.

Dense transformer block: qkv projection (+rank-4 LoRA on q and v),
16-head attention over [B=4, N=2048, C=1024], output projection + bias.

Sharding: tensor-parallel over heads across 8 NeuronCores. Each core owns
2 heads (128 of the 1024 channels of q/k/v and 128 rows of w_proj) and
computes a full [8192, 1024] partial of the output projection; the host
sums the 8 partials and adds the bias.

Device-side layout tricks (per core):
 - All matmul inputs bf16 (PE runs 4x slower on fp32); PSUM accumulates fp32.
 - LoRA is folded into the qkv weights on the host: x@A@B == x@(A@B).
 - Activations are kept feature-major (channel-on-partition, "xT") so no
   on-device transposes are ever needed:
     * q^T, k^T computed as [128ch, tok] slabs (2 heads stacked 64+64).
     * scores are computed TRANSPOSED: s^T[ktok, qtok] = k^T.T @ q^T via
       K=64 matmuls, both heads row-packed concurrently in the PE array
       (tile_position (0,0) and (64,0), separate PSUM banks).
     * softmax without max-subtraction (logits are O(3) here), exp on the
       scalar engine PSUM->SBUF bf16.
     * attn@v contracts over ktok with v stored token-major and augmented
       with a ones column, so the softmax denominator falls out of the
       same matmul (PSUM row 64).
     * normalization: reciprocal of the denominator row, partition-broadcast
       via two tiny K=1 matmuls, fused multiply during PSUM->SBUF eviction.
 - proj is computed token-major (out[tok,ch] = attn_outT.T @ w_proj) so the
   output DMA is contiguous.
"""

import sys

if '/opt/trn_rl_repo' not in sys.path:
    sys.path.insert(0, '/opt/trn_rl_repo')

import numpy as np
import ml_dtypes

import concourse.tile as tile
from concourse import bacc, mybir
from concourse.bass_utils import run_bass_kernel_spmd

BF16 = ml_dtypes.bfloat16

# Problem dims (hardcoded per contract)
B, N, C, H, D = 4, 2048, 1024, 16, 64
T = B * N                  # 8192 tokens
NCORES = 8
HC = H // NCORES           # 2 heads per core
LC = HC * D                # 128 local qkv columns / proj rows per core
KO = C // 128              # 8 contraction subtiles
SCALE = D ** -0.5          # 0.125
NKT = N // 128             # 16 ktok tiles per batch
DT = mybir.dt


def _build_nc():
    nc = bacc.Bacc(None, target_bir_lowering=False, debug=False)
    xT_d = nc.dram_tensor('xT', [C, T], DT.bfloat16, kind='ExternalInput')
    wq_d = nc.dram_tensor('wq', [C, LC], DT.bfloat16, kind='ExternalInput')
    wk_d = nc.dram_tensor('wk', [C, LC], DT.bfloat16, kind='ExternalInput')
    wv_d = nc.dram_tensor('wv', [C, LC], DT.bfloat16, kind='ExternalInput')
    wp_d = nc.dram_tensor('wp', [LC, C], DT.bfloat16, kind='ExternalInput')
    out_d = nc.dram_tensor('out', [T, C], DT.float32, kind='ExternalOutput')

    xT_r = xT_d.rearrange("(ko p) t -> p ko t", p=128)

    with tile.TileContext(nc) as tc:
        with (
            tc.tile_pool(name='weights', bufs=1) as cw,
            tc.tile_pool(name='xin', bufs=3) as xp,
            tc.tile_pool(name='slabs', bufs=1) as slabs,
            tc.tile_pool(name='work', bufs=1) as wk_pool,
            tc.tile_pool(name='ps', bufs=1, space='PSUM') as psp,
        ):
            wq_sb = cw.tile([128, KO, LC], DT.bfloat16)
            wk_sb = cw.tile([128, KO, LC], DT.bfloat16)
            wv_sb = cw.tile([128, KO, LC], DT.bfloat16)
            wp_sb = cw.tile([LC, C], DT.bfloat16)
            nc.sync.dma_start(wq_sb[:], wq_d.rearrange("(ko p) m -> p ko m", p=128))
            nc.sync.dma_start(wk_sb[:], wk_d.rearrange("(ko p) m -> p ko m", p=128))
            nc.sync.dma_start(wv_sb[:], wv_d.rearrange("(ko p) m -> p ko m", p=128))
            nc.sync.dma_start(wp_sb[:], wp_d[:])
            ones_row = cw.tile([1, 64], DT.bfloat16)
            nc.any.memset(ones_row[:], 1.0)

            # Fine-grained persistent tiles so the Tile scheduler sees exact
            # producer/consumer regions (one slab would serialize batches).
            q2T = [slabs.tile([128, 512], DT.bfloat16, name=f'q2T_{g}')
                   for g in range(T // 512)]
            k2T = [slabs.tile([128, 512], DT.bfloat16, name=f'k2T_{g}')
                   for g in range(T // 512)]
            aoT = [slabs.tile([128, 512], DT.bfloat16, name=f'aoT_{g}')
                   for g in range(T // 512)]
            vtok = [slabs.tile([128, 130], DT.bfloat16, name=f'vtok_{k}')
                    for k in range(T // 128)]
            for vt in vtok:
                nc.any.memset(vt[:, 64:65], 1.0)
                nc.any.memset(vt[:, 129:130], 1.0)

            for b in range(B):
                # ---- qkv + v (LoRA pre-folded into weights on host) ----
                for tt in range(4):
                    g = b * 4 + tt                      # global 512-tok tile
                    sl = slice(g * 512, (g + 1) * 512)
                    xt = xp.tile([128, KO, 512], DT.bfloat16, tag='xt')
                    nc.sync.dma_start(xt[:], xT_r[:, :, sl])

                    psq = psp.tile([128, 512], DT.float32, tag='aux', bufs=2)
                    for ko in range(KO):
                        nc.tensor.matmul(psq[:], wq_sb[:, ko, :], xt[:, ko, :],
                                         start=(ko == 0), stop=(ko == KO - 1))
                    nc.vector.tensor_scalar_mul(q2T[g][:], psq[:], SCALE)

                    psk = psp.tile([128, 512], DT.float32, tag='aux', bufs=2)
                    for ko in range(KO):
                        nc.tensor.matmul(psk[:], wk_sb[:, ko, :], xt[:, ko, :],
                                         start=(ko == 0), stop=(ko == KO - 1))
                    nc.vector.tensor_copy(k2T[g][:], psk[:])

                    for sub in range(4):
                        psv = psp.tile([128, 128], DT.float32, tag='aux', bufs=2)
                        tsl = slice(sub * 128, (sub + 1) * 128)
                        for ko in range(KO):
                            nc.tensor.matmul(psv[:], xt[:, ko, tsl], wv_sb[:, ko, :],
                                             start=(ko == 0), stop=(ko == KO - 1))
                        kt = g * 4 + sub
                        nc.vector.tensor_copy(vtok[kt][:, 0:64], psv[:, 0:64])
                        nc.vector.tensor_copy(vtok[kt][:, 65:129], psv[:, 64:128])

                # ---- attention (both heads row/col-packed) ----
                for qt in range(4):
                    qg = b * 4 + qt                     # 512-tok tile index
                    pso0 = psp.tile([128, 512], DT.float32, tag='o0', bufs=1)
                    pso1 = psp.tile([128, 512], DT.float32, tag='o1', bufs=1)
                    for kt16 in range(NKT):
                        ktg = b * NKT + kt16
                        kt_tile = k2T[ktg // 4]
                        ksl = slice((ktg % 4) * 128, (ktg % 4 + 1) * 128)
                        ss = psp.tile([128, 1024], DT.float32, tag='scores', bufs=2)
                        nc.tensor.matmul(ss[:, 0:512], kt_tile[0:64, ksl],
                                         q2T[qg][0:64, :], start=True, stop=True,
                                         tile_position=(0, 0))
                        nc.tensor.matmul(ss[:, 512:1024], kt_tile[64:128, ksl],
                                         q2T[qg][64:128, :], start=True, stop=True,
                                         tile_position=(64, 0))
                        es = wk_pool.tile([128, 1024], DT.bfloat16, tag='es', bufs=3)
                        nc.scalar.activation(es[:], ss[:],
                                             mybir.ActivationFunctionType.Exp)
                        nc.tensor.matmul(pso0[0:65, :], vtok[ktg][:, 0:65],
                                         es[:, 0:512],
                                         start=(kt16 == 0), stop=(kt16 == NKT - 1))
                        nc.tensor.matmul(pso1[0:65, :], vtok[ktg][:, 65:130],
                                         es[:, 512:1024],
                                         start=(kt16 == 0), stop=(kt16 == NKT - 1))
                    rr0 = wk_pool.tile([1, 512], DT.bfloat16, tag='rr0', bufs=2)
                    rr1 = wk_pool.tile([1, 512], DT.bfloat16, tag='rr1', bufs=2)
                    with nc.allow_low_precision(reason="bf16 softmax denominators"):
                        nc.vector.reciprocal(rr0[:], pso0[64:65, :])
                        nc.vector.reciprocal(rr1[:], pso1[64:65, :])
                    rbc = psp.tile([128, 512], DT.float32, tag='aux', bufs=2)
                    nc.tensor.matmul(rbc[0:64, :], ones_row[:], rr0[:],
                                     start=True, stop=True, tile_position=(0, 0))
                    nc.tensor.matmul(rbc[64:128, :], ones_row[:], rr1[:],
                                     start=True, stop=True, tile_position=(0, 64))
                    rbc_sb = wk_pool.tile([128, 512], DT.float32, tag='rbcsb', bufs=2)
                    nc.vector.tensor_copy(rbc_sb[:], rbc[:])
                    nc.vector.tensor_tensor(aoT[qg][0:64, :], pso0[0:64, :],
                                            rbc_sb[0:64, :], mybir.AluOpType.mult)
                    nc.vector.tensor_tensor(aoT[qg][64:128, :], pso1[0:64, :],
                                            rbc_sb[64:128, :], mybir.AluOpType.mult)

                # ---- output projection (token-major partial) ----
                for tt in range(16):
                    tg = b * 16 + tt                    # global 128-tok tile
                    tsl = slice(tg * 128, (tg + 1) * 128)
                    ao_tile = aoT[tg // 4]
                    asl = slice((tg % 4) * 128, (tg % 4 + 1) * 128)
                    for ch in range(2):
                        csl = slice(ch * 512, (ch + 1) * 512)
                        pspj = psp.tile([128, 512], DT.float32, tag='aux', bufs=2)
                        nc.tensor.matmul(pspj[:], ao_tile[:, asl], wp_sb[:, csl],
                                         start=True, stop=True)
                        ob = wk_pool.tile([128, 512], DT.float32, tag='ob', bufs=3)
                        nc.vector.tensor_copy(ob[:], pspj[:])
                        nc.sync.dma_start(out_d[tsl, csl], ob[:])

    nc.compile()
    return nc


def _prep_inputs(inputs):
    """Host-side sharding prep: returns per-core input maps."""
    x = np.asarray(inputs['x'], dtype=np.float32)
    w_qkv = np.asarray(inputs['w_qkv'], dtype=np.float32)
    w_a_q = np.asarray(inputs['w_a_q'], dtype=np.float32)
    w_b_q = np.asarray(inputs['w_b_q'], dtype=np.float32)
    w_a_v = np.asarray(inputs['w_a_v'], dtype=np.float32)
    w_b_v = np.asarray(inputs['w_b_v'], dtype=np.float32)
    w_proj = np.asarray(inputs['w_proj'], dtype=np.float32)

    wq_eff = w_qkv[:, :C] + w_a_q @ w_b_q
    wk_full = w_qkv[:, C:2 * C]
    wv_eff = w_qkv[:, 2 * C:] + w_a_v @ w_b_v

    xT = np.ascontiguousarray(x.reshape(T, C).T).astype(BF16)

    in_maps = []
    for m in range(NCORES):
        cols = slice(m * LC, (m + 1) * LC)
        in_maps.append({
            'xT': xT,
            'wq': np.ascontiguousarray(wq_eff[:, cols]).astype(BF16),
            'wk': np.ascontiguousarray(wk_full[:, cols]).astype(BF16),
            'wv': np.ascontiguousarray(wv_eff[:, cols]).astype(BF16),
            'wp': np.ascontiguousarray(w_proj[cols, :]).astype(BF16),
        })
    return in_maps


_nc_cache = None


def _get_nc():
    global _nc_cache
    if _nc_cache is None:
        _nc_cache = _build_nc()
    return _nc_cache


def kernel(**inputs) -> np.ndarray:
    nc = _get_nc()
    in_maps = _prep_inputs(inputs)
    res = run_bass_kernel_spmd(nc, in_maps, core_ids=list(range(NCORES)))
    b_proj = np.asarray(inputs['b_proj'], dtype=np.float32)
    total = res.results[0]['out'].astype(np.float32, copy=True)
    for m in range(1, NCORES):
        total += res.results[m]['out']
    total += b_proj[None, :]
    return total.reshape(B, N, C)


# revision 16
# speedup vs baseline: 246.2651x; 246.2651x over previous
"""Trainium2 Bass kernel for nn_Attention_27255862460439.

Dense transformer block: qkv projection (+rank-4 LoRA on q and v),
16-head attention over [B=4, N=2048, C=1024], output projection + bias.

Sharding: tensor-parallel over heads across 8 NeuronCores. Each core owns
2 heads (128 of the 1024 channels of q/k/v and 128 rows of w_proj) and
computes a full [8192, 1024] partial of the output projection; the host
sums the 8 partials and adds the bias.

Device-side design (per core):
 - All matmul inputs bf16 (PE runs 4x slower on fp32); PSUM accumulates fp32.
 - LoRA is folded into the qkv weights on the host: x@A@B == x@(A@B).
 - Activations are kept feature-major (channel-on-partition, "xT") so no
   on-device transposes are ever needed:
     * q^T, k^T computed as [128ch, tok] tiles (2 heads stacked 64+64).
     * scores are computed TRANSPOSED: s^T[ktok, qtok] = k^T.T @ q^T via
       K=64 matmuls, both heads row-packed in the PE array
       (tile_position (0,0) and (64,0), separate PSUM banks).
     * softmax without max-subtraction (logits are O(3) here), exp on the
       scalar engine PSUM->SBUF bf16.
     * attn@v contracts over ktok with v stored token-major and augmented
       with a ones column, so the softmax denominator falls out of the
       same matmul (PSUM row 64).
     * normalization: reciprocal of the denominator row, partition-broadcast
       via two tiny K=1 matmuls, fused multiply during PSUM->SBUF eviction.
 - proj is computed token-major (out[tok,ch] = attn_outT.T @ w_proj) so the
   output DMA is contiguous.
 - The attention inner loop is software-pipelined one kt ahead (the PE
   stream is in-order; scores(kt+1) must be emitted before attnV(kt) which
   waits on exp(kt)), and qkv work for batch b+1 / proj work for batch b-1
   is interleaved between scores and attnV as PE filler during the exp
   latency, so the scalar engine (the attention-phase bottleneck) never
   starves.
"""

import sys

if '/opt/trn_rl_repo' not in sys.path:
    sys.path.insert(0, '/opt/trn_rl_repo')

import numpy as np
import ml_dtypes

import concourse.tile as tile
from concourse import bacc, mybir
from concourse.bass_utils import run_bass_kernel_spmd

BF16 = ml_dtypes.bfloat16

# Problem dims (hardcoded per contract)
B, N, C, H, D = 4, 2048, 1024, 16, 64
T = B * N                  # 8192 tokens
NCORES = 8
HC = H // NCORES           # 2 heads per core
LC = HC * D                # 128 local qkv columns / proj rows per core
KO = C // 128              # 8 contraction subtiles
SCALE = D ** -0.5          # 0.125
NKT = N // 128             # 16 ktok tiles per batch
DT = mybir.dt


def _build_nc(loop_n: int = 1):
    nc = bacc.Bacc(None, target_bir_lowering=False, debug=False)
    xT_d = nc.dram_tensor('xT', [C, T], DT.bfloat16, kind='ExternalInput')
    wq_d = nc.dram_tensor('wq', [C, LC], DT.bfloat16, kind='ExternalInput')
    wk_d = nc.dram_tensor('wk', [C, LC], DT.bfloat16, kind='ExternalInput')
    wv_d = nc.dram_tensor('wv', [C, LC], DT.bfloat16, kind='ExternalInput')
    wp_d = nc.dram_tensor('wp', [LC, C], DT.bfloat16, kind='ExternalInput')
    out_d = nc.dram_tensor('out', [T, C], DT.float32, kind='ExternalOutput')

    xT_r = xT_d.rearrange("(ko p) t -> p ko t", p=128)

    with tile.TileContext(nc) as tc:
        with (
            tc.tile_pool(name='weights', bufs=1) as cw,
            tc.tile_pool(name='xin', bufs=6) as xp,
            tc.tile_pool(name='slabs', bufs=1) as slabs,
            tc.tile_pool(name='work', bufs=1) as wk_pool,
            tc.tile_pool(name='ps', bufs=1, space='PSUM') as psp,
        ):
            xt0 = cw.tile([128, KO, 512], DT.bfloat16, name='xt0')
            nc.sync.dma_start(xt0[:], xT_r[:, :, 0:512])
            wq_sb = cw.tile([128, KO, LC], DT.bfloat16)
            wk_sb = cw.tile([128, KO, LC], DT.bfloat16)
            wv_sb = cw.tile([128, KO, LC], DT.bfloat16)
            wp_sb = cw.tile([LC, C], DT.bfloat16)
            nc.sync.dma_start(wq_sb[:], wq_d.rearrange("(ko p) m -> p ko m", p=128))
            nc.sync.dma_start(wk_sb[:], wk_d.rearrange("(ko p) m -> p ko m", p=128))
            nc.sync.dma_start(wv_sb[:], wv_d.rearrange("(ko p) m -> p ko m", p=128))
            nc.sync.dma_start(wp_sb[:], wp_d[:])

            # Preload the exp table set while the first DMAs are in flight
            # (saves the ~2.7us ACT_TABLE_LOAD from the critical path).
            warm = cw.tile([1, 8], DT.float32)
            nc.any.memset(warm[:], 0.0)
            nc.scalar.activation(warm[:], warm[:],
                                 mybir.ActivationFunctionType.Exp)

            # Fine-grained persistent tiles (exact producer/consumer regions).
            q2T = [slabs.tile([128, 512], DT.bfloat16, name=f'q2T_{g}')
                   for g in range(T // 512)]
            k2T = [slabs.tile([128, 512], DT.bfloat16, name=f'k2T_{g}')
                   for g in range(T // 512)]
            aoT = [slabs.tile([128, 512], DT.bfloat16, name=f'aoT_{g}')
                   for g in range(T // 512)]
            vtok = [slabs.tile([128, 130], DT.bfloat16, name=f'vtok_{k}')
                    for k in range(T // 128)]
            for vt in vtok:
                nc.any.memset(vt[:, 64:65], 1.0)
                nc.any.memset(vt[:, 129:130], 1.0)

            def emit_body():
                # ---------- phase emitters ----------
                def qkv_tile_chunks(b, tt):
                    """Filler chunks (closures) computing q/k/v for one
                    512-token tile. Each chunk is a small burst of PE work."""
                    g = b * 4 + tt
                    sl = slice(g * 512, (g + 1) * 512)
                    state = {}

                    def load_x():
                        if g == 0:
                            state['xt'] = xt0
                            return
                        xt = xp.tile([128, KO, 512], DT.bfloat16, tag='xt')
                        nc.sync.dma_start(xt[:], xT_r[:, :, sl])
                        state['xt'] = xt

                    def q_mms(half):
                        if half == 0:
                            state['psq'] = psp.tile([128, 512], DT.float32,
                                                    tag='aux', bufs=2, name='psq')
                        psq, xt = state['psq'], state['xt']
                        for ko in range(half * 4, half * 4 + 4):
                            nc.tensor.matmul(psq[:], wq_sb[:, ko, :], xt[:, ko, :],
                                             start=(ko == 0), stop=(ko == KO - 1))
                        if half == 1:
                            nc.vector.tensor_scalar_mul(q2T[g][:], psq[:], SCALE)

                    def k_mms(half):
                        if half == 0:
                            state['psk'] = psp.tile([128, 512], DT.float32,
                                                    tag='aux', bufs=2, name='psk')
                        psk, xt = state['psk'], state['xt']
                        for ko in range(half * 4, half * 4 + 4):
                            nc.tensor.matmul(psk[:], wk_sb[:, ko, :], xt[:, ko, :],
                                             start=(ko == 0), stop=(ko == KO - 1))
                        if half == 1:
                            nc.vector.tensor_copy(k2T[g][:], psk[:])

                    def v_mms(sub):
                        xt = state['xt']
                        psv = psp.tile([128, 128], DT.float32, tag='aux', bufs=2,
                                       name='psv')
                        tsl = slice(sub * 128, (sub + 1) * 128)
                        for ko in range(KO):
                            nc.tensor.matmul(psv[:], xt[:, ko, tsl],
                                             wv_sb[:, ko, :],
                                             start=(ko == 0), stop=(ko == KO - 1))
                        kt = g * 4 + sub
                        # one strided copy: [128,2,64] -> cols {0:64, 65:129}
                        dst = vtok[kt].rearrange("p (two c) -> p two c", two=2)
                        src = psv[:].rearrange("p (two c) -> p two c", two=2)
                        nc.vector.tensor_copy(dst[:, :, 0:64], src)

                    chunks = [load_x,
                              lambda: q_mms(0), lambda: q_mms(1),
                              lambda: k_mms(0), lambda: k_mms(1)]
                    chunks += [(lambda s: lambda: v_mms(s))(s) for s in range(4)]
                    return chunks

                def proj_tile_chunk(b, tt, ch):
                    """One proj output tile: 1 matmul + evict + DMA out."""
                    tg = b * 16 + tt
                    tsl = slice(tg * 128, (tg + 1) * 128)
                    ao_tile = aoT[tg // 4]
                    asl = slice((tg % 4) * 128, (tg % 4 + 1) * 128)
                    csl = slice(ch * 512, (ch + 1) * 512)

                    def run():
                        pspj = psp.tile([128, 512], DT.float32, tag='aux', bufs=2,
                                        name='pspj')
                        nc.tensor.matmul(pspj[:], ao_tile[:, asl], wp_sb[:, csl],
                                         start=True, stop=True)
                        ob = wk_pool.tile([128, 512], DT.float32, tag='ob', bufs=3)
                        nc.vector.tensor_copy(ob[:], pspj[:])
                        nc.sync.dma_start(out_d[tsl, csl], ob[:])
                    return run

                def emit_scores(b, qt, kt16):
                    ktg = b * NKT + kt16
                    kt_tile = k2T[ktg // 4]
                    ksl = slice((ktg % 4) * 128, (ktg % 4 + 1) * 128)
                    ss = psp.tile([128, 1024], DT.float32, tag='scores', bufs=2,
                                  name='ss')
                    nc.tensor.matmul(ss[:, 0:512], kt_tile[0:64, ksl],
                                     q2T[b * 4 + qt][0:64, :],
                                     start=True, stop=True, tile_position=(0, 0))
                    nc.tensor.matmul(ss[:, 512:1024], kt_tile[64:128, ksl],
                                     q2T[b * 4 + qt][64:128, :],
                                     start=True, stop=True, tile_position=(64, 0))
                    return ss

                # ---------- prologue: qkv for batch 0, tile 0 ----------
                for c in qkv_tile_chunks(0, 0):
                    c()

                # ---------- attention per batch, with filler interleave ----
                # Fillers are (deadline_tile_or_None, closure). Deadlined
                # chunks (batch-0 qkv tiles 1..3) must be EMITTED before the
                # scores that read their outputs; the rest are paced evenly.
                for b in range(B):
                    fillers = []
                    if b == 0:
                        for tt in range(1, 4):
                            for c in qkv_tile_chunks(0, tt):
                                fillers.append((tt, c))
                    if b + 1 < B:
                        for tt in range(4):
                            for c in qkv_tile_chunks(b + 1, tt):
                                fillers.append((None, c))
                    if b > 0:
                        # leftover proj chunks from batch b-1 (its qt=3)
                        for tt in range(12, 16):
                            for ch in range(2):
                                fillers.append((None, proj_tile_chunk(b - 1, tt, ch)))
                    fillers.reverse()          # pop() takes from the front

                    n_iters = 4 * NKT
                    it = 0
                    if b == 0:
                        ss_next = emit_scores(0, 0, 0)
                    for qt in range(4):
                        pso0 = psp.tile([128, 512], DT.float32, tag='o0', bufs=1,
                                        name='pso0')
                        pso1 = psp.tile([128, 512], DT.float32, tag='o1', bufs=1,
                                        name='pso1')
                        for kt16 in range(NKT):
                            ktg = b * NKT + kt16
                            # mandatory flush: producers of the tile the
                            # upcoming scores emission will read
                            next_tile = min(qt * NKT + kt16 + 1, n_iters - 1) // 16
                            next_kt_tile = (kt16 + 1) // 4 if kt16 < NKT - 1 else 0
                            while fillers and fillers[-1][0] is not None and \
                                    fillers[-1][0] <= max(next_kt_tile, next_tile):
                                fillers.pop()[1]()
                            ss = ss_next
                            if kt16 < NKT - 1:
                                ss_next = emit_scores(b, qt, kt16 + 1)
                            elif qt < 3:
                                ss_next = emit_scores(b, qt + 1, 0)
                            elif b + 1 < B:
                                ss_next = emit_scores(b + 1, 0, 0)
                            else:
                                ss_next = None
                            es = wk_pool.tile([128, 1024], DT.bfloat16, tag='es',
                                              bufs=3)
                            nc.scalar.activation(es[:], ss[:],
                                                 mybir.ActivationFunctionType.Exp)
                            # paced filler PE work rides out the exp latency
                            remaining = n_iters - it
                            want = -(-len(fillers) // remaining)
                            for _ in range(min(want, 3)):
                                if fillers:
                                    fillers.pop()[1]()
                            it += 1
                            nc.tensor.matmul(pso0[0:65, :], vtok[ktg][:, 0:65],
                                             es[:, 0:512],
                                             start=(kt16 == 0),
                                             stop=(kt16 == NKT - 1))
                            nc.tensor.matmul(pso1[0:65, :], vtok[ktg][:, 65:130],
                                             es[:, 512:1024],
                                             start=(kt16 == 0),
                                             stop=(kt16 == NKT - 1))
                        # normalize qt: fp32 reciprocal of the denominator
                        # rows, partition-broadcast on the (idle) GpSimd
                        # engine, then divide-as-multiply during eviction.
                        rr0 = wk_pool.tile([1, 512], DT.float32, tag='rr0',
                                           bufs=2)
                        rr1 = wk_pool.tile([1, 512], DT.float32, tag='rr1',
                                           bufs=2)
                        nc.vector.reciprocal(rr0[:], pso0[64:65, :])
                        nc.vector.reciprocal(rr1[:], pso1[64:65, :])
                        # partition_broadcast only writes correctly to
                        # base-0 targets -> use two separate [64,512] tiles
                        rbc0 = wk_pool.tile([64, 512], DT.float32, tag='rbc0',
                                            bufs=2)
                        rbc1 = wk_pool.tile([64, 512], DT.float32, tag='rbc1',
                                            bufs=2)
                        nc.gpsimd.partition_broadcast(rbc0[:], rr0[:])
                        nc.gpsimd.partition_broadcast(rbc1[:], rr1[:])
                        qg = b * 4 + qt
                        nc.vector.tensor_tensor(aoT[qg][0:64, :], pso0[0:64, :],
                                                rbc0[:], mybir.AluOpType.mult)
                        nc.vector.tensor_tensor(aoT[qg][64:128, :], pso1[0:64, :],
                                                rbc1[:], mybir.AluOpType.mult)
                        # this qt's proj work becomes filler for later qts
                        if qt < 3:
                            for tt in range(qt * 4, qt * 4 + 4):
                                for ch in range(2):
                                    fillers.insert(
                                        0, (None, proj_tile_chunk(b, tt, ch)))
                    # leftover fillers
                    while fillers:
                        fillers.pop()[1]()

                # ---------- epilogue: proj for last batch's qt=3 ----------
                for tt in range(12, 16):
                    for ch in range(2):
                        proj_tile_chunk(B - 1, tt, ch)()

            if loop_n > 1:
                with tc.For_i(0, loop_n, 1):
                    emit_body()
            else:
                emit_body()

    nc.compile()
    return nc


def _prep_inputs(inputs):
    """Host-side sharding prep: returns per-core input maps."""
    x = np.asarray(inputs['x'], dtype=np.float32)
    w_qkv = np.asarray(inputs['w_qkv'], dtype=np.float32)
    w_a_q = np.asarray(inputs['w_a_q'], dtype=np.float32)
    w_b_q = np.asarray(inputs['w_b_q'], dtype=np.float32)
    w_a_v = np.asarray(inputs['w_a_v'], dtype=np.float32)
    w_b_v = np.asarray(inputs['w_b_v'], dtype=np.float32)
    w_proj = np.asarray(inputs['w_proj'], dtype=np.float32)

    wq_eff = w_qkv[:, :C] + w_a_q @ w_b_q
    wk_full = w_qkv[:, C:2 * C]
    wv_eff = w_qkv[:, 2 * C:] + w_a_v @ w_b_v

    xT = np.ascontiguousarray(x.reshape(T, C).T).astype(BF16)

    in_maps = []
    for m in range(NCORES):
        cols = slice(m * LC, (m + 1) * LC)
        in_maps.append({
            'xT': xT,
            'wq': np.ascontiguousarray(wq_eff[:, cols]).astype(BF16),
            'wk': np.ascontiguousarray(wk_full[:, cols]).astype(BF16),
            'wv': np.ascontiguousarray(wv_eff[:, cols]).astype(BF16),
            'wp': np.ascontiguousarray(w_proj[cols, :]).astype(BF16),
        })
    return in_maps


_nc_cache = None


def _get_nc():
    global _nc_cache
    if _nc_cache is None:
        _nc_cache = _build_nc()
    return _nc_cache


def kernel(**inputs) -> np.ndarray:
    nc = _get_nc()
    in_maps = _prep_inputs(inputs)
    res = run_bass_kernel_spmd(nc, in_maps, core_ids=list(range(NCORES)))
    b_proj = np.asarray(inputs['b_proj'], dtype=np.float32)
    total = res.results[0]['out'].astype(np.float32, copy=True)
    for m in range(1, NCORES):
        total += res.results[m]['out']
    total += b_proj[None, :]
    return total.reshape(B, N, C)


# revision 20
# speedup vs baseline: 263.8332x; 1.0713x over previous
"""Trainium2 Bass kernel for nn_Attention_27255862460439.

Dense transformer block: qkv projection (+rank-4 LoRA on q and v),
16-head attention over [B=4, N=2048, C=1024], output projection + bias.

Sharding: tensor-parallel over heads across 8 NeuronCores. Each core owns
2 heads (128 of the 1024 channels of q/k/v and 128 rows of w_proj) and
computes a full [8192, 1024] partial of the output projection; the host
sums the 8 partials and adds the bias.

Device-side design (per core):
 - All matmul inputs bf16 (PE runs 4x slower on fp32); PSUM accumulates fp32.
 - LoRA is folded into the qkv weights on the host: x@A@B == x@(A@B).
 - Activations are kept feature-major (channel-on-partition, "xT") so no
   on-device transposes are ever needed:
     * q^T, k^T computed as [128ch, tok] tiles (2 heads stacked 64+64).
     * scores are computed TRANSPOSED: s^T[ktok, qtok] = k^T.T @ q^T via
       K=64 matmuls, both heads row-packed in the PE array
       (tile_position (0,0) and (64,0), separate PSUM banks).
     * softmax without max-subtraction (logits are O(3) here), exp on the
       scalar engine PSUM->SBUF bf16.
     * attn@v contracts over ktok with v stored token-major and augmented
       with a ones column, so the softmax denominator falls out of the
       same matmul (PSUM row 64).
     * normalization: reciprocal of the denominator row, partition-broadcast
       via two tiny K=1 matmuls, fused multiply during PSUM->SBUF eviction.
 - proj is computed token-major (out[tok,ch] = attn_outT.T @ w_proj) so the
   output DMA is contiguous.
 - The attention inner loop is software-pipelined one kt ahead (the PE
   stream is in-order; scores(kt+1) must be emitted before attnV(kt) which
   waits on exp(kt)), and qkv work for batch b+1 / proj work for batch b-1
   is interleaved between scores and attnV as PE filler during the exp
   latency, so the scalar engine (the attention-phase bottleneck) never
   starves.
"""

import sys

if '/opt/trn_rl_repo' not in sys.path:
    sys.path.insert(0, '/opt/trn_rl_repo')

import numpy as np
import ml_dtypes

import concourse.tile as tile
from concourse import bacc, mybir
from concourse.bass_utils import run_bass_kernel_spmd

BF16 = ml_dtypes.bfloat16

# Problem dims (hardcoded per contract)
B, N, C, H, D = 4, 2048, 1024, 16, 64
T = B * N                  # 8192 tokens
NCORES = 8
HC = H // NCORES           # 2 heads per core
LC = HC * D                # 128 local qkv columns / proj rows per core
KO = C // 128              # 8 contraction subtiles
SCALE = D ** -0.5          # 0.125
NKT = N // 128             # 16 ktok tiles per batch
DT = mybir.dt


def _build_nc(loop_n: int = 1):
    nc = bacc.Bacc(None, target_bir_lowering=False, debug=False)
    xT_d = nc.dram_tensor('xT', [C, T], DT.bfloat16, kind='ExternalInput')
    wq_d = nc.dram_tensor('wq', [128, KO, LC], DT.bfloat16, kind='ExternalInput')
    wk_d = nc.dram_tensor('wk', [128, KO, LC], DT.bfloat16, kind='ExternalInput')
    wv_d = nc.dram_tensor('wv', [128, KO, LC], DT.bfloat16, kind='ExternalInput')
    wp_d = nc.dram_tensor('wp', [LC, C], DT.bfloat16, kind='ExternalInput')
    out_d = nc.dram_tensor('out', [T, C], DT.float32, kind='ExternalOutput')

    xT_r = xT_d.rearrange("(ko p) t -> p ko t", p=128)

    with tile.TileContext(nc) as tc:
        with (
            tc.tile_pool(name='weights', bufs=1) as cw,
            tc.tile_pool(name='xin', bufs=6) as xp,
            tc.tile_pool(name='slabs', bufs=1) as slabs,
            tc.tile_pool(name='work', bufs=1) as wk_pool,
            tc.tile_pool(name='ps', bufs=1, space='PSUM') as psp,
        ):
            xt0 = cw.tile([128, KO, 512], DT.bfloat16, name='xt0')
            nc.sync.dma_start(xt0[:], xT_r[:, :, 0:512])
            wq_sb = cw.tile([128, KO, LC], DT.bfloat16)
            wk_sb = cw.tile([128, KO, LC], DT.bfloat16)
            wv_sb = cw.tile([128, KO, LC], DT.bfloat16)
            wp_sb = cw.tile([LC, C], DT.bfloat16)
            nc.sync.dma_start(wq_sb[:], wq_d[:])
            nc.sync.dma_start(wk_sb[:], wk_d[:])
            nc.sync.dma_start(wv_sb[:], wv_d[:])
            nc.sync.dma_start(wp_sb[:], wp_d[:])

            # Preload the exp table set while the first DMAs are in flight
            # (saves the ~2.7us ACT_TABLE_LOAD from the critical path).
            warm = cw.tile([1, 8], DT.float32)
            nc.any.memset(warm[:], 0.0)
            nc.scalar.activation(warm[:], warm[:],
                                 mybir.ActivationFunctionType.Exp)

            # Fine-grained persistent tiles (exact producer/consumer regions).
            q2T = [slabs.tile([128, 512], DT.bfloat16, name=f'q2T_{g}')
                   for g in range(T // 512)]
            k2T = [slabs.tile([128, 512], DT.bfloat16, name=f'k2T_{g}')
                   for g in range(T // 512)]
            aoT = [slabs.tile([128, 512], DT.bfloat16, name=f'aoT_{g}')
                   for g in range(T // 512)]
            vtok = [slabs.tile([128, 130], DT.bfloat16, name=f'vtok_{k}')
                    for k in range(T // 128)]
            for vt in vtok:
                nc.any.memset(vt[:, 64:65], 1.0)
                nc.any.memset(vt[:, 129:130], 1.0)

            def emit_body():
                # ---------- phase emitters ----------
                def qkv_tile_chunks(b, tt):
                    """Filler chunks (closures) computing q/k/v for one
                    512-token tile. Each chunk is a small burst of PE work."""
                    g = b * 4 + tt
                    sl = slice(g * 512, (g + 1) * 512)
                    state = {}

                    def load_x():
                        if g == 0:
                            state['xt'] = xt0
                            return
                        xt = xp.tile([128, KO, 512], DT.bfloat16, tag='xt')
                        nc.sync.dma_start(xt[:], xT_r[:, :, sl])
                        state['xt'] = xt

                    def q_mms(half):
                        if half == 0:
                            state['psq'] = psp.tile([128, 512], DT.float32,
                                                    tag='aux', bufs=2, name='psq')
                        psq, xt = state['psq'], state['xt']
                        for ko in range(half * 4, half * 4 + 4):
                            nc.tensor.matmul(psq[:], wq_sb[:, ko, :], xt[:, ko, :],
                                             start=(ko == 0), stop=(ko == KO - 1))
                        if half == 1:
                            nc.vector.tensor_scalar_mul(q2T[g][:], psq[:], SCALE)

                    def k_mms(half):
                        if half == 0:
                            state['psk'] = psp.tile([128, 512], DT.float32,
                                                    tag='aux', bufs=2, name='psk')
                        psk, xt = state['psk'], state['xt']
                        for ko in range(half * 4, half * 4 + 4):
                            nc.tensor.matmul(psk[:], wk_sb[:, ko, :], xt[:, ko, :],
                                             start=(ko == 0), stop=(ko == KO - 1))
                        if half == 1:
                            nc.vector.tensor_copy(k2T[g][:], psk[:])

                    def v_mms(sub):
                        xt = state['xt']
                        psv = psp.tile([128, 128], DT.float32, tag='aux', bufs=2,
                                       name='psv')
                        tsl = slice(sub * 128, (sub + 1) * 128)
                        for ko in range(KO):
                            nc.tensor.matmul(psv[:], xt[:, ko, tsl],
                                             wv_sb[:, ko, :],
                                             start=(ko == 0), stop=(ko == KO - 1))
                        kt = g * 4 + sub
                        # one strided copy: [128,2,64] -> cols {0:64, 65:129}
                        dst = vtok[kt].rearrange("p (two c) -> p two c", two=2)
                        src = psv[:].rearrange("p (two c) -> p two c", two=2)
                        nc.vector.tensor_copy(dst[:, :, 0:64], src)

                    chunks = [load_x,
                              lambda: q_mms(0), lambda: q_mms(1),
                              lambda: k_mms(0), lambda: k_mms(1)]
                    chunks += [(lambda s: lambda: v_mms(s))(s) for s in range(4)]
                    return chunks

                def proj_tile_chunk(b, tt, ch):
                    """One proj output tile: 1 matmul + evict + DMA out."""
                    tg = b * 16 + tt
                    tsl = slice(tg * 128, (tg + 1) * 128)
                    ao_tile = aoT[tg // 4]
                    asl = slice((tg % 4) * 128, (tg % 4 + 1) * 128)
                    csl = slice(ch * 512, (ch + 1) * 512)

                    tail = (b == B - 1 and tt >= 12)

                    def run():
                        pspj = psp.tile([128, 512], DT.float32, tag='aux', bufs=2,
                                        name='pspj')
                        nc.tensor.matmul(pspj[:], ao_tile[:, asl], wp_sb[:, csl],
                                         start=True, stop=True)
                        ob = wk_pool.tile([128, 512], DT.float32, tag='ob', bufs=3)
                        # at the kernel tail ACT is idle; split evictions
                        if tail and ch == 1:
                            nc.scalar.copy(ob[:], pspj[:])
                        else:
                            nc.vector.tensor_copy(ob[:], pspj[:])
                        nc.sync.dma_start(out_d[tsl, csl], ob[:])
                    return run

                def emit_scores(b, qt, kt16):
                    ktg = b * NKT + kt16
                    kt_tile = k2T[ktg // 4]
                    ksl = slice((ktg % 4) * 128, (ktg % 4 + 1) * 128)
                    ss = psp.tile([128, 1024], DT.float32, tag='scores', bufs=2,
                                  name='ss')
                    nc.tensor.matmul(ss[:, 0:512], kt_tile[0:64, ksl],
                                     q2T[b * 4 + qt][0:64, :],
                                     start=True, stop=True, tile_position=(0, 0))
                    nc.tensor.matmul(ss[:, 512:1024], kt_tile[64:128, ksl],
                                     q2T[b * 4 + qt][64:128, :],
                                     start=True, stop=True, tile_position=(64, 0))
                    return ss

                # ---------- prologue: qkv for batch 0, tile 0 ----------
                for c in qkv_tile_chunks(0, 0):
                    c()

                # ---------- attention per batch, with filler interleave ----
                # Fillers are (deadline_tile_or_None, closure). Deadlined
                # chunks (batch-0 qkv tiles 1..3) must be EMITTED before the
                # scores that read their outputs; the rest are paced evenly.
                for b in range(B):
                    fillers = []
                    if b == 0:
                        for tt in range(1, 4):
                            for c in qkv_tile_chunks(0, tt):
                                fillers.append((tt, c))
                    if b + 1 < B:
                        for tt in range(4):
                            for c in qkv_tile_chunks(b + 1, tt):
                                fillers.append((None, c))
                    if b > 0:
                        # leftover proj chunks from batch b-1 (its qt=3)
                        for tt in range(12, 16):
                            for ch in range(2):
                                fillers.append((None, proj_tile_chunk(b - 1, tt, ch)))
                    fillers.reverse()          # pop() takes from the front

                    n_iters = 4 * NKT
                    it = 0
                    if b == 0:
                        ss_next = emit_scores(0, 0, 0)
                    for qt in range(4):
                        pso0 = psp.tile([128, 512], DT.float32, tag='o0', bufs=1,
                                        name='pso0')
                        pso1 = psp.tile([128, 512], DT.float32, tag='o1', bufs=1,
                                        name='pso1')
                        for kt16 in range(NKT):
                            ktg = b * NKT + kt16
                            # mandatory flush: producers of the tile the
                            # upcoming scores emission will read
                            next_tile = min(qt * NKT + kt16 + 1, n_iters - 1) // 16
                            next_kt_tile = (kt16 + 1) // 4 if kt16 < NKT - 1 else 0
                            while fillers and fillers[-1][0] is not None and \
                                    fillers[-1][0] <= max(next_kt_tile, next_tile):
                                fillers.pop()[1]()
                            ss = ss_next
                            if kt16 < NKT - 1:
                                ss_next = emit_scores(b, qt, kt16 + 1)
                            elif qt < 3:
                                ss_next = emit_scores(b, qt + 1, 0)
                            elif b + 1 < B:
                                ss_next = emit_scores(b + 1, 0, 0)
                            else:
                                ss_next = None
                            es = wk_pool.tile([128, 1024], DT.bfloat16, tag='es',
                                              bufs=3)
                            nc.scalar.activation(es[:], ss[:],
                                                 mybir.ActivationFunctionType.Exp)
                            # paced filler PE work rides out the exp latency
                            remaining = n_iters - it
                            if fillers and fillers[-1][0] is not None:
                                # deadlined (batch-0) work: pace to land just
                                # ahead of its consumers instead of bursting
                                # at the mandatory-flush point
                                remaining = max(1, min(remaining,
                                                       4 * fillers[-1][0] - it))
                            want = -(-len(fillers) // remaining)
                            for _ in range(min(want, 3)):
                                if fillers:
                                    fillers.pop()[1]()
                            it += 1
                            nc.tensor.matmul(pso0[0:65, :], vtok[ktg][:, 0:65],
                                             es[:, 0:512],
                                             start=(kt16 == 0),
                                             stop=(kt16 == NKT - 1))
                            nc.tensor.matmul(pso1[0:65, :], vtok[ktg][:, 65:130],
                                             es[:, 512:1024],
                                             start=(kt16 == 0),
                                             stop=(kt16 == NKT - 1))
                        # normalize qt: evacuate the accumulators to SBUF
                        # first (frees the PSUM banks for the next qt's
                        # attnV as early as possible), then do the whole
                        # normalize SBUF->SBUF where DVE runs in 2x mode.
                        u0 = wk_pool.tile([65, 512], DT.float32, tag='u0',
                                          bufs=2)
                        u1 = wk_pool.tile([65, 512], DT.float32, tag='u1',
                                          bufs=2)
                        nc.vector.tensor_copy(u0[:], pso0[0:65, :])
                        nc.vector.tensor_copy(u1[:], pso1[0:65, :])
                        rr0 = wk_pool.tile([1, 512], DT.float32, tag='rr0',
                                           bufs=2)
                        rr1 = wk_pool.tile([1, 512], DT.float32, tag='rr1',
                                           bufs=2)
                        nc.vector.reciprocal(rr0[:], u0[64:65, :])
                        nc.vector.reciprocal(rr1[:], u1[64:65, :])
                        # partition_broadcast only writes correctly to
                        # base-0 targets -> use two separate [64,512] tiles
                        rbc0 = wk_pool.tile([64, 512], DT.float32, tag='rbc0',
                                            bufs=2)
                        rbc1 = wk_pool.tile([64, 512], DT.float32, tag='rbc1',
                                            bufs=2)
                        nc.gpsimd.partition_broadcast(rbc0[:], rr0[:])
                        nc.gpsimd.partition_broadcast(rbc1[:], rr1[:])
                        qg = b * 4 + qt
                        nc.vector.tensor_tensor(aoT[qg][0:64, :], u0[0:64, :],
                                                rbc0[:], mybir.AluOpType.mult)
                        nc.vector.tensor_tensor(aoT[qg][64:128, :], u1[0:64, :],
                                                rbc1[:], mybir.AluOpType.mult)
                        # this qt's proj work becomes filler for later qts
                        if qt < 3:
                            for tt in range(qt * 4, qt * 4 + 4):
                                for ch in range(2):
                                    fillers.insert(
                                        0, (None, proj_tile_chunk(b, tt, ch)))
                    # leftover fillers
                    while fillers:
                        fillers.pop()[1]()

                # ---------- epilogue: proj for last batch's qt=3 ----------
                for tt in range(12, 16):
                    for ch in range(2):
                        proj_tile_chunk(B - 1, tt, ch)()

            if loop_n > 1:
                with tc.For_i(0, loop_n, 1):
                    emit_body()
            else:
                emit_body()

    nc.compile()
    return nc


def _prep_inputs(inputs):
    """Host-side sharding prep: returns per-core input maps."""
    x = np.asarray(inputs['x'], dtype=np.float32)
    w_qkv = np.asarray(inputs['w_qkv'], dtype=np.float32)
    w_a_q = np.asarray(inputs['w_a_q'], dtype=np.float32)
    w_b_q = np.asarray(inputs['w_b_q'], dtype=np.float32)
    w_a_v = np.asarray(inputs['w_a_v'], dtype=np.float32)
    w_b_v = np.asarray(inputs['w_b_v'], dtype=np.float32)
    w_proj = np.asarray(inputs['w_proj'], dtype=np.float32)

    wq_eff = w_qkv[:, :C] + w_a_q @ w_b_q
    wk_full = w_qkv[:, C:2 * C]
    wv_eff = w_qkv[:, 2 * C:] + w_a_v @ w_b_v

    xT = np.ascontiguousarray(x.reshape(T, C).T).astype(BF16)

    in_maps = []
    for m in range(NCORES):
        cols = slice(m * LC, (m + 1) * LC)
        def pack(w):
            # [C, LC] -> [p, ko, m] so the device DMA is one contiguous blob
            return np.ascontiguousarray(
                w.reshape(KO, 128, LC).transpose(1, 0, 2)).astype(BF16)
        in_maps.append({
            'xT': xT,
            'wq': pack(wq_eff[:, cols]),
            'wk': pack(wk_full[:, cols]),
            'wv': pack(wv_eff[:, cols]),
            'wp': np.ascontiguousarray(w_proj[cols, :]).astype(BF16),
        })
    return in_maps


_nc_cache = None


def _get_nc():
    global _nc_cache
    if _nc_cache is None:
        _nc_cache = _build_nc()
    return _nc_cache


def kernel(**inputs) -> np.ndarray:
    nc = _get_nc()
    in_maps = _prep_inputs(inputs)
    res = run_bass_kernel_spmd(nc, in_maps, core_ids=list(range(NCORES)))
    b_proj = np.asarray(inputs['b_proj'], dtype=np.float32)
    total = res.results[0]['out'].astype(np.float32, copy=True)
    for m in range(1, NCORES):
        total += res.results[m]['out']
    total += b_proj[None, :]
    return total.reshape(B, N, C)
